# revision 1
# baseline (speedup 1.0000x reference)
"""Distributed Trainium2 Bass kernel for causal multi-head attention with RoPE.

Reference computation (B=2, S=2048, E=1024, H=16, D=64, fp32):
    q = rope((x @ Wq.T).heads); k = rope((x @ Wk.T).heads); v = (x @ Wv.T).heads
    out = softmax(mask(q k^T / sqrt(E))) v  -> concat heads -> @ Wo.T

Sharding (8 NeuronCores): data parallel over B (2 groups of 4 cores),
tensor parallel over heads within each group (4 heads per core).
Each core computes QKV for its 4 heads, flash-style causal attention,
normalized attention output transposed (d x s). A chunked AllGather
(4-rank groups, one chunk per 512-column sq block) concatenates the
per-head attention outputs while later chunks still compute; every core
then computes a 256-column slice of the final Wo projection per chunk.

Host-side prep (per-core input shards):
  - x fed transposed (E,S) in bf16.
  - Wq/Wk rows permuted per head to de-interleave RoPE pairs (even dims
    first, odd dims second) so RoPE becomes the rotate-half form.
  - cos/sin tables and the 32-row swap matrix are precomputed constants.
"""

import os
import sys

sys.path.insert(0, "/opt/trn_rl_repo")

import numpy as np
import ml_dtypes

import concourse.bass as bass
import concourse.bacc as bacc
import concourse.mybir as mybir
import concourse.tile as tile
from concourse import bass_utils

B, S, E, H, D = 2, 2048, 1024, 16, 64
NCORES = 8
TP = 4                 # tensor-parallel group size
HPC = H // TP          # heads per core = 4
DQ = HPC * D           # per-core projection width = 256
ATTN_SCALE = 1.0 / float(np.sqrt(E))

FP32 = mybir.dt.float32
BF16 = mybir.dt.bfloat16

SQT = 512              # sq tile (free dim of S^T tiles)
SKB = 128              # sk block (partition dim of S^T tiles)
NSQT = S // SQT        # 4
NST16 = S // 128       # 16
NE = E // 128          # 8 contraction steps

REPLICA_GROUPS = [[0, 1, 2, 3], [4, 5, 6, 7]]

_CACHE = {}
LAST_RESULT = None


def build_nc():
    nc = bacc.Bacc(None, target_bir_lowering=False)

    xT = nc.declare_dram_parameter("xT", [E, S], BF16, isOutput=False)
    wqT = nc.declare_dram_parameter("wqT", [E, DQ], BF16, isOutput=False)
    wkT = nc.declare_dram_parameter("wkT", [E, DQ], BF16, isOutput=False)
    wvT = nc.declare_dram_parameter("wvT", [E, DQ], BF16, isOutput=False)
    woT = nc.declare_dram_parameter("woT", [E, DQ], BF16, isOutput=False)
    cosd = nc.declare_dram_parameter("cos", [128, S], FP32, isOutput=False)
    sind = nc.declare_dram_parameter("sin", [128, S], FP32, isOutput=False)
    swapd = nc.declare_dram_parameter("swapmat", [128, 128], BF16, isOutput=False)
    out_ext = nc.declare_dram_parameter("out", [S, DQ], FP32, isOutput=True)

    with tile.TileContext(nc) as tc:
        with (
            tc.tile_pool(name="dram", bufs=1, space="DRAM") as drampool,
            tc.tile_pool(name="const", bufs=1) as constpool,
        ):
            # ---- persistent SBUF tensors; DMA order gates pipeline start ----
            w_sb = {}
            for name in ("wq", "wk", "wv", "wo"):
                w_sb[name] = constpool.tile(
                    [128, NE * DQ], BF16, tag=f"w_{name}", name=f"w_{name}"
                )

            def load_w(name, dram):
                for j in range(NE):
                    nc.sync.dma_start(
                        out=w_sb[name][:, j * DQ:(j + 1) * DQ],
                        in_=dram[j * 128:(j + 1) * 128, :],
                    )

            cos_sb = constpool.tile([128, S], FP32, tag="cos")
            sin_sb = constpool.tile([128, S], FP32, tag="sin")
            swap_sb = constpool.tile([128, 128], BF16, tag="swap")

            qt_sb = [
                constpool.tile([128, S], BF16, tag=f"qt{g}", name=f"qt{g}")
                for g in range(2)
            ]
            kt_sb = [
                constpool.tile([128, S], BF16, tag=f"kt{g}", name=f"kt{g}")
                for g in range(2)
            ]
            vaug = [
                constpool.tile([128, HPC * 65], BF16, tag=f"vaug{i}", name=f"vaug{i}")
                for i in range(NST16)
            ]
            attnT = [
                constpool.tile([64, S], BF16, tag=f"attn{h}", name=f"attn{h}")
                for h in range(HPC)
            ]

            # ---------------- Phase 1: QKV projections + RoPE ----------------
            with (
                tc.tile_pool(name="xt", bufs=1) as xtpool,
                tc.tile_pool(name="ps1", bufs=2, space="PSUM") as ps1pool,
                tc.tile_pool(name="ps2", bufs=2, space="PSUM") as ps2pool,
                tc.tile_pool(name="psv", bufs=2, space="PSUM") as psvpool,
                tc.tile_pool(name="ropetmp", bufs=3) as rtpool,
            ):
                xt = [
                    xtpool.tile([128, S], BF16, tag=f"xT{j}", name=f"xT{j}")
                    for j in range(NE)
                ]
                # gate-critical loads first
                load_w("wq", wqT)
                for j in range(NE):
                    nc.sync.dma_start(out=xt[j][:], in_=xT[j * 128:(j + 1) * 128, :])
                load_w("wk", wkT)
                nc.sync.dma_start(out=swap_sb[:], in_=swapd[:])
                nc.sync.dma_start(out=cos_sb[:], in_=cosd[:])
                nc.sync.dma_start(out=sin_sb[:], in_=sind[:])
                load_w("wv", wvT)
                load_w("wo", woT)

                # V projection first: attention needs vaug tiles from kb=0,
                # so emitting V early keeps the attention pipeline unblocked.
                for i in range(NST16):
                    psv = psvpool.tile([128, DQ], FP32, tag="psv")
                    for j in range(NE):
                        nc.tensor.matmul(
                            psv[:],
                            lhsT=xt[j][:, i * 128:(i + 1) * 128],
                            rhs=w_sb["wv"][:, j * DQ:(j + 1) * DQ],
                            start=(j == 0),
                            stop=(j == NE - 1),
                        )
                    nc.gpsimd.memset(vaug[i][:], 1.0)
                    # one strided copy drops V into the 4 per-head 65-wide
                    # slots, leaving column 64 of each slot at 1.0
                    nc.scalar.copy(
                        vaug[i][:, 0:HPC * 65].rearrange(
                            "p (h w) -> p h w", h=HPC
                        )[:, :, 0:64],
                        psv[:].rearrange("p (h w) -> p h w", h=HPC),
                    )

                for g in range(2):
                    for st in range(NSQT):
                        sq = slice(st * SQT, (st + 1) * SQT)
                        for wname, dst in (("wq", qt_sb), ("wk", kt_sb)):
                            ps = ps1pool.tile([128, SQT], FP32, tag="ps")
                            for j in range(NE):
                                nc.tensor.matmul(
                                    ps[:],
                                    lhsT=w_sb[wname][
                                        :, j * DQ + g * 128: j * DQ + g * 128 + 128
                                    ],
                                    rhs=xt[j][:, sq],
                                    start=(j == 0),
                                    stop=(j == NE - 1),
                                )
                            raw = rtpool.tile([128, SQT], BF16, tag="raw")
                            nc.scalar.copy(raw[:], ps[:])
                            ps_sw = ps2pool.tile([128, SQT], FP32, tag="ps_sw")
                            nc.tensor.matmul(
                                ps_sw[:], lhsT=swap_sb[:], rhs=raw[:],
                                start=True, stop=True,
                            )
                            t1 = rtpool.tile([128, SQT], FP32, tag="t1")
                            nc.vector.tensor_mul(t1[:], ps_sw[:], sin_sb[:, sq])
                            t2 = rtpool.tile([128, SQT], FP32, tag="t2")
                            nc.vector.tensor_mul(t2[:], raw[:], cos_sb[:, sq])
                            nc.vector.tensor_add(dst[g][:, sq], t1[:], t2[:])

            # ------- Phase 2+3: causal attention, chunked AG, Wo -------
            with (
                tc.tile_pool(name="pss", bufs=2, space="PSUM") as psspool,
                tc.tile_pool(name="pso", bufs=1, space="PSUM") as psopool,
                tc.tile_pool(name="psw", bufs=2, space="PSUM") as pswpool,
                tc.tile_pool(name="pt", bufs=3) as ptpool,
                tc.tile_pool(name="fin", bufs=2) as finpool,
                tc.tile_pool(name="gt", bufs=2) as gtpool,
                tc.tile_pool(name="osb", bufs=3) as osbpool,
            ):
                # sq chunks; the last 512 block is split so the tail-exposed
                # final AllGather is half size. Parity psum slots always sit
                # at 0/512-f32 offsets (separate PSUM banks) — the two
                # concurrently-issued parity matmuls must never share a bank
                # (fatal PSUM collision).
                CHUNKS = [(0, 512), (512, 512), (1024, 512), (1536, 512)]

                def wo_block(sq0, cw, gt):
                    for i4 in range(cw // 128):
                        r0 = sq0 + i4 * 128
                        psw = pswpool.tile(
                            [128, DQ], FP32, tag="psw", name=f"psw{r0}"
                        )
                        for j in range(NE):
                            nc.tensor.matmul(
                                psw[:],
                                lhsT=gt[j][:, i4 * 128:(i4 + 1) * 128],
                                rhs=w_sb["wo"][:, j * DQ:(j + 1) * DQ],
                                start=(j == 0),
                                stop=(j == NE - 1),
                            )
                        osb = osbpool.tile(
                            [128, DQ], FP32, tag="osb", name=f"osb{r0}"
                        )
                        nc.vector.tensor_copy(osb[:], psw[:])
                        nc.sync.dma_start(
                            out=out_ext[r0:r0 + 128, :], in_=osb[:]
                        )

                wo_queue = []    # (sq0, cw, gt tiles) ready for Wo
                ag_pending = []  # (sq0, cw, agout) awaiting gt load
                for ci, (sq0, cw) in enumerate(CHUNKS):
                    sq = slice(sq0, sq0 + cw)
                    nblk = (sq0 + cw) // SKB
                    for g in range(2):
                        pso = [
                            psopool.tile([65, cw], FP32, tag=f"pso{p}",
                                         name=f"pso{p}_{g}_{ci}")
                            for p in range(2)
                        ]
                        for kb in range(nblk):
                            pss = psspool.tile([SKB, 2 * SQT], FP32, tag="pss",
                                               name=f"pss_{g}_{ci}_{kb}")
                            for p in range(2):
                                nc.tensor.matmul(
                                    pss[:, p * SQT: p * SQT + cw],
                                    lhsT=kt_sb[g][
                                        p * 64:(p + 1) * 64, kb * SKB:(kb + 1) * SKB
                                    ],
                                    rhs=qt_sb[g][p * 64:(p + 1) * 64, sq],
                                    start=True,
                                    stop=True,
                                )
                            pt = ptpool.tile([SKB, 2 * SQT], BF16, tag="pt",
                                             name=f"pt_{g}_{ci}_{kb}")
                            diag = (kb + 1) * SKB > sq0
                            if cw == SQT:
                                nc.scalar.activation(
                                    pt[:], pss[:],
                                    mybir.ActivationFunctionType.Exp,
                                    scale=ATTN_SCALE,
                                )
                                if diag:
                                    nc.gpsimd.affine_select(
                                        out=pt[:],
                                        in_=pt[:],
                                        compare_op=mybir.AluOpType.is_ge,
                                        fill=0.0,
                                        base=sq0 - kb * SKB,
                                        channel_multiplier=-1,
                                        pattern=[[0, 2], [1, cw]],
                                    )
                            else:
                                for p in range(2):
                                    sl = slice(p * SQT, p * SQT + cw)
                                    nc.scalar.activation(
                                        pt[:, sl], pss[:, sl],
                                        mybir.ActivationFunctionType.Exp,
                                        scale=ATTN_SCALE,
                                    )
                                    if diag:
                                        nc.gpsimd.affine_select(
                                            out=pt[:, sl],
                                            in_=pt[:, sl],
                                            compare_op=mybir.AluOpType.is_ge,
                                            fill=0.0,
                                            base=sq0 - kb * SKB,
                                            channel_multiplier=-1,
                                            pattern=[[1, cw]],
                                        )
                            for p in range(2):
                                h = 2 * g + p
                                nc.tensor.matmul(
                                    pso[p][:],
                                    lhsT=vaug[kb][:, h * 65:(h + 1) * 65],
                                    rhs=pt[:, p * SQT: p * SQT + cw],
                                    start=(kb == 0),
                                    stop=(kb == nblk - 1),
                                )
                        # evacuate both pso tiles FIRST so their PSUM slots
                        # free ~0.5us after the last PV — the slow reciprocal
                        # chain then runs off the PE-critical path.
                        un = []
                        lrow = []
                        for p in range(2):
                            u = finpool.tile([64, cw], FP32, tag=f"un{p}",
                                             name=f"un{p}_{g}_{ci}")
                            nc.vector.tensor_copy(u[:], pso[p][0:64, :])
                            lr = finpool.tile([1, cw], FP32, tag=f"lrow{p}",
                                              name=f"lrow{p}_{g}_{ci}")
                            nc.vector.tensor_copy(lr[:], pso[p][64:65, :])
                            un.append(u)
                            lrow.append(lr)
                        for p in range(2):
                            h = 2 * g + p
                            linv = finpool.tile([1, cw], FP32, tag=f"linv{p}")
                            nc.vector.reciprocal(linv[:], lrow[p][:])
                            lbc = finpool.tile([64, cw], FP32, tag=f"lbc{p}")
                            nc.gpsimd.partition_broadcast(lbc[:], linv[:])
                            nc.vector.tensor_mul(
                                attnT[h][:, sq], un[p][:], lbc[:]
                            )

                    # ---- AllGather this sq chunk ----
                    agin = drampool.tile(
                        [DQ, cw], BF16, tag=f"agin{ci}", name=f"agin{ci}"
                    )
                    agout = drampool.tile(
                        [E, cw], BF16, tag=f"agout{ci}", name=f"agout{ci}"
                    )
                    for h in range(HPC):
                        nc.sync.dma_start(
                            out=agin[h * 64:(h + 1) * 64, :], in_=attnT[h][:, sq]
                        )
                    nc.gpsimd.collective_compute(
                        "AllGather",
                        mybir.AluOpType.bypass,
                        ins=[agin.opt()],
                        outs=[agout.opt()],
                        replica_groups=REPLICA_GROUPS,
                    )
                    # Wo runs with a two-chunk lag and its gathered tiles are
                    # DMA'd one chunk after their AllGather was issued: the gt
                    # trigger's CC wait is then already satisfied, so it never
                    # clogs the in-order Sync queue (which would delay the
                    # next chunk's agin DMA and cascade AG delays).
                    if wo_queue:
                        wo_block(*wo_queue.pop(0))
                    if ag_pending:
                        psq0, pcw, pagout = ag_pending.pop(0)
                        gt = []
                        for j in range(NE):
                            t = gtpool.tile(
                                [128, pcw], BF16, tag=f"gt{j}",
                                name=f"gt{j}_{psq0}"
                            )
                            nc.sync.dma_start(
                                out=t[:], in_=pagout[j * 128:(j + 1) * 128, :]
                            )
                            gt.append(t)
                        wo_queue.append((psq0, pcw, gt))
                    ag_pending.append((sq0, cw, agout))
                while ag_pending:
                    psq0, pcw, pagout = ag_pending.pop(0)
                    gt = []
                    for j in range(NE):
                        t = gtpool.tile(
                            [128, pcw], BF16, tag=f"gt{j}", name=f"gt{j}_{psq0}"
                        )
                        nc.sync.dma_start(
                            out=t[:], in_=pagout[j * 128:(j + 1) * 128, :]
                        )
                        gt.append(t)
                    wo_queue.append((psq0, pcw, gt))
                while wo_queue:
                    wo_block(*wo_queue.pop(0))

    nc.finalize()
    return nc


def _host_tables():
    inv = 1.0 / (10000.0 ** (np.arange(0, D, 2, dtype=np.float64) / D))  # (32,)
    ang = np.arange(S, dtype=np.float64)[None, :] * inv[:, None]          # (32,S)
    cos32 = np.cos(ang)
    sin32 = np.sin(ang)
    cos = np.tile(cos32, (4, 1)).astype(np.float32)                       # (128,S)
    sin = np.concatenate([-sin32, sin32, -sin32, sin32], axis=0).astype(np.float32)
    swap = np.zeros((128, 128), np.float32)
    for k in range(128):
        blk = (k // 64) * 64
        swap[k, blk + ((k - blk) + 32) % 64] = 1.0
    return cos, sin, swap


def kernel(x, W_q, W_k, W_v, W_o):
    global LAST_RESULT
    if "nc" not in _CACHE:
        _CACHE["nc"] = build_nc()
    nc = _CACHE["nc"]

    bf = ml_dtypes.bfloat16
    perm = np.concatenate([np.arange(0, D, 2), np.arange(1, D, 2)])
    rowperm = (np.arange(H)[:, None] * D + perm[None, :]).reshape(-1)
    Wq_p = W_q[rowperm]
    Wk_p = W_k[rowperm]
    cos, sin, swap = _host_tables()
    swap_bf = swap.astype(bf)

    in_maps = []
    for c in range(NCORES):
        b, tp = c // TP, c % TP
        sl = slice(tp * DQ, (tp + 1) * DQ)
        in_maps.append({
            "xT": np.ascontiguousarray(x[b].T).astype(bf),
            "wqT": np.ascontiguousarray(Wq_p[sl].T).astype(bf),
            "wkT": np.ascontiguousarray(Wk_p[sl].T).astype(bf),
            "wvT": np.ascontiguousarray(W_v[sl].T).astype(bf),
            "woT": np.ascontiguousarray(W_o[sl].T).astype(bf),
            "cos": cos,
            "sin": sin,
            "swapmat": swap_bf,
        })

    res = bass_utils.run_bass_kernel_spmd(
        nc, in_maps, core_ids=list(range(NCORES)),
        tmpdir=os.environ.get("BASS_TMPDIR") or None,
    )
    LAST_RESULT = res
    out = np.empty((B, S, E), np.float32)
    for c in range(NCORES):
        b, tp = c // TP, c % TP
        out[b][:, tp * DQ:(tp + 1) * DQ] = np.asarray(
            res.results[c]["out"], dtype=np.float32
        )
    return out



# revision 3
# speedup vs baseline: 1.8529x; 1.8529x over previous
"""Distributed Trainium2 Bass kernel for causal multi-head attention with RoPE.

Reference computation (B=2, S=2048, E=1024, H=16, D=64, fp32):
    q = rope((x @ Wq.T).heads); k = rope((x @ Wk.T).heads); v = (x @ Wv.T).heads
    out = softmax(mask(q k^T / sqrt(E))) v  -> concat heads -> @ Wo.T

Sharding (8 NeuronCores): data parallel over B (2 groups of 4 cores),
tensor parallel over heads within each group (4 heads per core).
Each core computes QKV for its 4 heads, flash-style causal attention,
then a ROW-PARALLEL Wo partial product: out_partial = attn_own @ Wo.T[own,:]
giving a full-width (S, E) bf16 partial per core.  The host sums the 4
partials per batch group (the unshard step) -- no device collective.

On-core schedule: V projection first (dense PE warm-up), then per
512-query chunk st: QK projection + RoPE for chunk st, causal attention
for chunk st (keys 0..(st+1)*512), and the Wo partial for chunk st.
This interleaves PE-heavy projection with ScalarE-heavy softmax so both
engines stay busy.  Scores are issued one k-block ahead of PV so the PE
never stalls on the exp.  Diagonal k-blocks are tail-sliced: only the
causally-valid column tail is exp'd / masked / PV'd.

Host-side prep (per-core input shards):
  - x fed transposed (E,S) in bf16.
  - Wq/Wk rows permuted per head to de-interleave RoPE pairs (even dims
    first, odd dims second) so RoPE becomes the rotate-half form.
  - cos/sin tables (bf16) and the 32-row swap matrix are constants.
"""

import os
import sys

sys.path.insert(0, "/opt/trn_rl_repo")

import numpy as np
import ml_dtypes

import concourse.bass as bass
import concourse.bacc as bacc
import concourse.mybir as mybir
import concourse.tile as tile
from concourse import bass_utils

B, S, E, H, D = 2, 2048, 1024, 16, 64
NCORES = 8
TP = 4                 # tensor-parallel group size
HPC = H // TP          # heads per core = 4
DQ = HPC * D           # per-core projection width = 256
ATTN_SCALE = 1.0 / float(np.sqrt(E))

FP32 = mybir.dt.float32
BF16 = mybir.dt.bfloat16

SQT = 512              # query chunk width
SKB = 128              # key block (partition dim of score tiles)
NSQT = S // SQT        # 4
NST16 = S // 128       # 16
NE = E // 128          # 8 contraction steps

_CACHE = {}
LAST_RESULT = None


def build_nc():
    nc = bacc.Bacc(None, target_bir_lowering=False)

    xT = nc.declare_dram_parameter("xT", [E, S], BF16, isOutput=False)
    wqT = nc.declare_dram_parameter("wqT", [E, DQ], BF16, isOutput=False)
    wkT = nc.declare_dram_parameter("wkT", [E, DQ], BF16, isOutput=False)
    wvT = nc.declare_dram_parameter("wvT", [E, DQ], BF16, isOutput=False)
    woT = nc.declare_dram_parameter("woT", [DQ, E], BF16, isOutput=False)
    cosd = nc.declare_dram_parameter("cos", [128, S], BF16, isOutput=False)
    sind = nc.declare_dram_parameter("sin", [128, S], BF16, isOutput=False)
    swapd = nc.declare_dram_parameter("swapmat", [128, 128], BF16, isOutput=False)
    out_ext = nc.declare_dram_parameter("out", [S, E], BF16, isOutput=True)

    with tile.TileContext(nc) as tc:
        with tc.tile_pool(name="const", bufs=1) as constpool:
            # ---- persistent SBUF tensors; DMA order gates pipeline start ----
            w_sb = {}
            for name in ("wq", "wk", "wv"):
                w_sb[name] = constpool.tile(
                    [128, NE * DQ], BF16, tag=f"w_{name}", name=f"w_{name}"
                )
            # wo: 2 contraction blocks (128 own-dims each) x full E columns
            wo_sb = constpool.tile([128, 2 * E], BF16, tag="w_wo", name="w_wo")

            cos_sb = constpool.tile([128, S], BF16, tag="cos")
            sin_sb = constpool.tile([128, S], BF16, tag="sin")
            swap_sb = constpool.tile([128, 128], BF16, tag="swap")

            xt = [
                constpool.tile([128, S], BF16, tag=f"xT{j}", name=f"xT{j}")
                for j in range(NE)
            ]

            qt_sb = [
                constpool.tile([128, S], BF16, tag=f"qt{g}", name=f"qt{g}")
                for g in range(2)
            ]
            kt_sb = [
                constpool.tile([128, S], BF16, tag=f"kt{g}", name=f"kt{g}")
                for g in range(2)
            ]
            vaug = [
                constpool.tile([128, HPC * 65], BF16, tag=f"vaug{i}", name=f"vaug{i}")
                for i in range(NST16)
            ]
            # attention outputs (normalized), head-pair stacked: attnT[j]
            # holds heads 2j (rows 0-63) and 2j+1 (rows 64-127), (d, s).
            attnT = [
                constpool.tile([128, S], BF16, tag=f"attn{j}", name=f"attn{j}")
                for j in range(2)
            ]

            # gate-critical loads first: x and wv feed the V projection.
            for j in range(NE):
                nc.sync.dma_start(out=xt[j][:], in_=xT[j * 128:(j + 1) * 128, :])
            for name, dram in (("wv", wvT), ("wq", wqT), ("wk", wkT)):
                for j in range(NE):
                    nc.sync.dma_start(
                        out=w_sb[name][:, j * DQ:(j + 1) * DQ],
                        in_=dram[j * 128:(j + 1) * 128, :],
                    )
            nc.sync.dma_start(out=swap_sb[:], in_=swapd[:])
            nc.sync.dma_start(out=cos_sb[:], in_=cosd[:])
            nc.sync.dma_start(out=sin_sb[:], in_=sind[:])
            for j in range(2):
                nc.sync.dma_start(
                    out=wo_sb[:, j * E:(j + 1) * E],
                    in_=woT[j * 128:(j + 1) * 128, :],
                )

            # ---------------- Phase V: V projection -> vaug ----------------
            with tc.tile_pool(name="psv", bufs=4, space="PSUM") as psvpool:
                for i in range(NST16):
                    psv = psvpool.tile([128, DQ], FP32, tag="psv", name=f"psv{i}")
                    for j in range(NE):
                        nc.tensor.matmul(
                            psv[:],
                            lhsT=xt[j][:, i * 128:(i + 1) * 128],
                            rhs=w_sb["wv"][:, j * DQ:(j + 1) * DQ],
                            start=(j == 0),
                            stop=(j == NE - 1),
                        )
                    nc.gpsimd.memset(vaug[i][:], 1.0)
                    # one strided copy drops V into the 4 per-head 65-wide
                    # slots, leaving column 64 of each slot at 1.0
                    nc.scalar.copy(
                        vaug[i][:, 0:HPC * 65].rearrange(
                            "p (h w) -> p h w", h=HPC
                        )[:, :, 0:64],
                        psv[:].rearrange("p (h w) -> p h w", h=HPC),
                    )

            # ------- Phase 2: per-chunk QK+RoPE, attention, Wo partial -------
            with (
                tc.tile_pool(name="pss", bufs=2, space="PSUM") as psspool,
                tc.tile_pool(name="ps2", bufs=1, space="PSUM") as ps2pool,
                tc.tile_pool(name="po", bufs=3, space="PSUM") as popool,
                tc.tile_pool(name="pt", bufs=3) as ptpool,
                tc.tile_pool(name="rope", bufs=2) as rtpool,
                tc.tile_pool(name="fin", bufs=2) as finpool,
                tc.tile_pool(name="osb", bufs=2) as osbpool,
            ):
                for st in range(NSQT):
                    sq0 = st * SQT
                    sq = slice(sq0, sq0 + SQT)

                    # ---- QK projection + RoPE for queries/keys in chunk ----
                    rope_tail = []
                    for g in range(2):
                        for wname, dst in (("wq", qt_sb), ("wk", kt_sb)):
                            ps = psspool.tile(
                                [128, 2 * SQT], FP32, tag="ps",
                                name=f"ps_{wname}_{g}_{st}",
                            )
                            for j in range(NE):
                                nc.tensor.matmul(
                                    ps[:, 0:SQT],
                                    lhsT=w_sb[wname][
                                        :, j * DQ + g * 128: j * DQ + g * 128 + 128
                                    ],
                                    rhs=xt[j][:, sq],
                                    start=(j == 0),
                                    stop=(j == NE - 1),
                                )
                            raw = rtpool.tile([128, SQT], BF16, tag="raw",
                                              name=f"raw_{wname}_{g}_{st}")
                            nc.scalar.copy(raw[:], ps[:, 0:SQT])
                            ps_sw = ps2pool.tile([128, SQT], FP32, tag="ps_sw",
                                                 name=f"ps_sw_{wname}_{g}_{st}")
                            nc.tensor.matmul(
                                ps_sw[:], lhsT=swap_sb[:], rhs=raw[:],
                                start=True, stop=True,
                            )
                            rope_tail.append((g, dst, raw, ps_sw))
                    for g, dst, raw, ps_sw in rope_tail:
                        t1 = rtpool.tile([128, SQT], BF16, tag="t1")
                        nc.vector.tensor_mul(t1[:], ps_sw[:], sin_sb[:, sq])
                        t2 = rtpool.tile([128, SQT], BF16, tag="t2")
                        nc.vector.tensor_mul(t2[:], raw[:], cos_sb[:, sq])
                        nc.vector.tensor_add(dst[g][:, sq], t1[:], t2[:])

                    # ---- causal attention for chunk st ----
                    nblk = (sq0 + SQT) // SKB
                    for g in range(2):
                        pso = [
                            popool.tile([128, SQT], FP32, tag="po",
                                        name=f"pso{p}_{g}_{st}")
                            for p in range(2)
                        ]

                        def issue_scores(kb):
                            c0 = max(0, kb * SKB - sq0)
                            ps = psspool.tile(
                                [128, 2 * SQT], FP32, tag="ps",
                                name=f"pss_{g}_{st}_{kb}",
                            )
                            for p in range(2):
                                nc.tensor.matmul(
                                    ps[:, p * SQT + c0:(p + 1) * SQT],
                                    lhsT=kt_sb[g][
                                        p * 64:(p + 1) * 64,
                                        kb * SKB:(kb + 1) * SKB,
                                    ],
                                    rhs=qt_sb[g][
                                        p * 64:(p + 1) * 64, sq0 + c0:sq0 + SQT
                                    ],
                                    start=True,
                                    stop=True,
                                )
                            return ps, c0

                        def pv_tail(kb, ps, c0):
                            w = SQT - c0
                            pt = ptpool.tile([128, 2 * SQT], BF16, tag="pt",
                                             name=f"pt_{g}_{st}_{kb}")
                            if c0 == 0:
                                nc.scalar.activation(
                                    pt[:], ps[:],
                                    mybir.ActivationFunctionType.Exp,
                                    scale=ATTN_SCALE,
                                )
                            else:
                                psview = ps[:].rearrange(
                                    "p (h w) -> p h w", h=2)[:, :, c0:]
                                ptview = pt[:].rearrange(
                                    "p (h w) -> p h w", h=2)[:, :, c0:]
                                nc.scalar.activation(
                                    ptview, psview,
                                    mybir.ActivationFunctionType.Exp,
                                    scale=ATTN_SCALE,
                                )
                            if kb * SKB >= sq0:  # diagonal block: mask tail
                                ptview = pt[:].rearrange(
                                    "p (h w) -> p h w", h=2)[:, :, c0:]
                                nc.gpsimd.affine_select(
                                    out=ptview,
                                    in_=ptview,
                                    compare_op=mybir.AluOpType.is_ge,
                                    fill=0.0,
                                    base=0,
                                    channel_multiplier=-1,
                                    pattern=[[0, 2], [1, w]],
                                )
                            for p in range(2):
                                h = 2 * g + p
                                nc.tensor.matmul(
                                    pso[p][0:65, c0:SQT],
                                    lhsT=vaug[kb][:, h * 65:(h + 1) * 65],
                                    rhs=pt[:, p * SQT + c0:(p + 1) * SQT],
                                    start=(kb == 0),
                                    stop=(kb == nblk - 1),
                                )

                        # scores one block ahead of PV so ACT latency hides
                        pending = {0: issue_scores(0)}
                        if nblk > 1:
                            pending[1] = issue_scores(1)
                        for kb in range(nblk):
                            ps, c0 = pending.pop(kb)
                            pv_tail(kb, ps, c0)
                            if kb + 2 < nblk:
                                pending[kb + 2] = issue_scores(kb + 2)

                        for p in range(2):
                            lrow = finpool.tile([1, SQT], FP32, tag=f"lrow{p}")
                            nc.vector.tensor_copy(lrow[:], pso[p][64:65, :])
                            linv = finpool.tile([1, SQT], FP32, tag=f"linv{p}")
                            nc.vector.reciprocal_approx_fast(
                                linv[:], lrow[:]
                            )
                            lbc = finpool.tile([64, SQT], FP32, tag=f"lbc{p}")
                            nc.gpsimd.partition_broadcast(lbc[:], linv[:])
                            nc.vector.tensor_mul(
                                attnT[g][p * 64:(p + 1) * 64, sq],
                                pso[p][0:64, :],
                                lbc[:],
                            )

                    # ---- Wo partial for chunk st ----
                    for i4 in range(4):
                        r0 = sq0 + i4 * 128
                        osb = osbpool.tile([128, E], BF16, tag="osb",
                                           name=f"osb{r0}")
                        for nh in range(2):
                            psw = popool.tile([128, 512], FP32, tag="po",
                                              name=f"psw_{r0}_{nh}")
                            for j in range(2):
                                nc.tensor.matmul(
                                    psw[:],
                                    lhsT=attnT[j][:, r0:r0 + 128],
                                    rhs=wo_sb[
                                        :, j * E + nh * 512: j * E + nh * 512 + 512
                                    ],
                                    start=(j == 0),
                                    stop=(j == 1),
                                )
                            nc.vector.tensor_copy(
                                osb[:, nh * 512:(nh + 1) * 512], psw[:]
                            )
                        nc.sync.dma_start(
                            out=out_ext[r0:r0 + 128, :], in_=osb[:]
                        )

    nc.finalize()
    return nc


def _host_tables():
    inv = 1.0 / (10000.0 ** (np.arange(0, D, 2, dtype=np.float64) / D))  # (32,)
    ang = np.arange(S, dtype=np.float64)[None, :] * inv[:, None]          # (32,S)
    cos32 = np.cos(ang)
    sin32 = np.sin(ang)
    cos = np.tile(cos32, (4, 1)).astype(np.float32)                       # (128,S)
    sin = np.concatenate([-sin32, sin32, -sin32, sin32], axis=0).astype(np.float32)
    swap = np.zeros((128, 128), np.float32)
    for k in range(128):
        blk = (k // 64) * 64
        swap[k, blk + ((k - blk) + 32) % 64] = 1.0
    return cos, sin, swap


def kernel(x, W_q, W_k, W_v, W_o):
    global LAST_RESULT
    if "nc" not in _CACHE:
        _CACHE["nc"] = build_nc()
    nc = _CACHE["nc"]

    bf = ml_dtypes.bfloat16
    perm = np.concatenate([np.arange(0, D, 2), np.arange(1, D, 2)])
    rowperm = (np.arange(H)[:, None] * D + perm[None, :]).reshape(-1)
    Wq_p = W_q[rowperm]
    Wk_p = W_k[rowperm]
    cos, sin, swap = _host_tables()

    in_maps = []
    for c in range(NCORES):
        b, tp = c // TP, c % TP
        sl = slice(tp * DQ, (tp + 1) * DQ)
        in_maps.append({
            "xT": np.ascontiguousarray(x[b].T).astype(bf),
            "wqT": np.ascontiguousarray(Wq_p[sl].T).astype(bf),
            "wkT": np.ascontiguousarray(Wk_p[sl].T).astype(bf),
            "wvT": np.ascontiguousarray(W_v[sl].T).astype(bf),
            # row-parallel Wo: rows of Wo.T for this core's attn dims
            "woT": np.ascontiguousarray(W_o[:, sl].T).astype(bf),
            "cos": cos.astype(bf),
            "sin": sin.astype(bf),
            "swapmat": swap.astype(bf),
        })

    res = bass_utils.run_bass_kernel_spmd(
        nc, in_maps, core_ids=list(range(NCORES)),
        tmpdir=os.environ.get("BASS_TMPDIR") or None,
    )
    LAST_RESULT = res
    out = np.zeros((B, S, E), np.float32)
    for c in range(NCORES):
        b = c // TP
        out[b] += np.asarray(res.results[c]["out"], dtype=np.float32)
    return out
